# revision 1
# baseline (speedup 1.0000x reference)
"""Cost-volume layer (17-shift cross pattern, R=4) for Trainium2, 8 NeuronCores.

out[b,s,h,w] = sum_c src[b,c,h,w] * tgt[b,c,h+dh_s,w+dw_s]   (tgt zero-padded)

Strategy
--------
Shard: 8 cores = batch(4) x H-halves(2). Per core: src [128, 48*160],
tgt (padded, with halo) [128, 56*168]. C=128 lives in the SBUF partition
dim and is contracted on the TensorEngine via *banded correlations*:

- vertical shifts  (dh=-4..4, dw=0): per column w, matmul
    src[:, :, w]^T @ tgt[:, :, w+4]  ->  [48, 56] band matrix
- horizontal shifts (dh=0, dw=-4..4): per row h, per 32-col chunk i, matmul
    src[:, h, 32i:32i+32]^T @ tgt[:, h+4, 32i:32i+40] -> [32, 40] band matrix

Useful entries are the 9 diagonals of each band; bands are packed into PSUM
banks (several chunks per bank at 32-aligned partition bases), staged to SBUF
(DVE/ACT copies), DMA'd to HBM, and the diagonals are gathered host-side
(pure indexing - no host arithmetic).

Input loads are sliced by row-groups so horizontal banks start while the
tail of the inputs is still loading; band writes go out on the ACT HWDGE
ring so they do not serialize against input loads on the sync ring.
"""

import numpy as np
from contextlib import ExitStack

import concourse.bacc as bacc
import concourse.tile as tile
from concourse import mybir
from concourse import bass_utils

R = 4
B, C, H, W = 4, 128, 96, 160
NCORES = 8
HSH = H // 2            # 48 output rows per shard
HT = HSH + 2 * R        # 56 tgt rows (with halo)
WP = W + 2 * R          # 168 padded width
F32 = mybir.dt.float32

# compute dtype for the matmul inputs. float16 runs the PE at full rate
# (1 cyc/row vs fp32's 4) and halves input DMA, at ~3e-4 relative error
# (randn inputs are far from fp16 overflow). "float32" is the exact fallback.
COMPUTE_DT = "float16"

# vertical pass: per-w matmul M=48, N=56; pack 2 groups (part base 0, 64)
# x 9 w-slots per PSUM bank -> 18 w per bank
VSLOT = 9
VBASES = (0, 64)
VPERBANK = VSLOT * len(VBASES)          # 18
NVBANK = (W + VPERBANK - 1) // VPERBANK  # 9

# horizontal pass: chunks of 32 src cols, window N=40; pack 4 groups
# (bases 0,32,64,96) x 12 slots per bank -> 48 chunks per bank
MH = 32
NH = MH + 2 * R         # 40
NCH = W // MH           # 5 chunks per row
NQ = HSH * NCH          # 240 chunks total
HSLOT = 12
HBASES = (0, 32, 64, 96)
HPERBANK = HSLOT * len(HBASES)          # 48
NHBANK = (NQ + HPERBANK - 1) // HPERBANK  # 5

SHIFTS = [(0, 0)]
for i in range(1, R + 1):
    SHIFTS.extend([(-i, 0), (i, 0), (0, -i), (0, i)])

# input load row-slices: tgt rows [0,16,32,44,56), src rows [0,12,24,36,48)
TGT_CUTS = [0, 16, 32, 44, 56]
SRC_CUTS = [0, 12, 24, 36, 48]
# horizontal bank b covers h in [b*48/5, ...]; bank ready after these pieces:
#   bank0: h<=9  -> tgt rows <=13 (piece 1), src rows <=9  (piece 1)
#   bank1: h<=19 -> tgt <=23 (piece 2), src <=19 (piece 2)
#   bank2: h<=28 -> tgt <=32 (piece 3), src <=28 (piece 3)
#   bank3: h<=38 -> tgt <=42 (piece 3), src <=38 (piece 4)
#   bank4: h<=47 -> all


def build_nc():
    cdt = getattr(mybir.dt, COMPUTE_DT)
    bdt = mybir.dt.float16 if COMPUTE_DT != "float32" else F32
    nc = bacc.Bacc("TRN2", target_bir_lowering=False)
    src = nc.dram_tensor("src", [C, HSH * W], cdt, kind="ExternalInput")
    tgt = nc.dram_tensor("tgt", [C, HT * WP], cdt, kind="ExternalInput")
    # band layouts are DMA-run-friendly: vband is flushed with ONE dma per
    # 3-bank group spanning all 112 stage partitions (hole rows memset by
    # the otherwise-idle GPSIMD), so every DMA port is active with 3KB runs
    vband = nc.dram_tensor("vband", [3, 112, 3, VSLOT * HT], bdt,
                           kind="ExternalOutput")
    hband = nc.dram_tensor("hband", [MH, len(HBASES), NHBANK * HSLOT, NH], bdt,
                           kind="ExternalOutput")

    with ExitStack() as ctx:
        tc = ctx.enter_context(tile.TileContext(nc))
        ins = ctx.enter_context(tc.tile_pool(name="ins", bufs=1))
        psum = ctx.enter_context(tc.tile_pool(name="psum", bufs=4, space="PSUM"))
        stage = ctx.enter_context(tc.tile_pool(name="stage", bufs=6))

        src_sb = ins.tile([C, HSH * W], cdt)
        tgt_sb = ins.tile([C, HT * WP], cdt)

        def load_piece(i):
            t0, t1 = TGT_CUTS[i] * WP, TGT_CUTS[i + 1] * WP
            s0, s1 = SRC_CUTS[i] * W, SRC_CUTS[i + 1] * W
            nc.sync.dma_start(out=tgt_sb[:, t0:t1], in_=tgt[:][:, t0:t1])
            nc.sync.dma_start(out=src_sb[:, s0:s1], in_=src[:][:, s0:s1])

        src3 = src_sb.rearrange("c (h w) -> c h w", w=W)
        tgt3 = tgt_sb.rearrange("c (h w) -> c h w", w=WP)

        copy_flip = [0]

        def stage_copy(dst, src_ap):
            # alternate PSUM->SBUF copies between DVE and ACT
            if copy_flip[0] % 2 == 0:
                nc.vector.tensor_copy(out=dst, in_=src_ap)
            else:
                nc.scalar.copy(out=dst, in_=src_ap)
            copy_flip[0] += 1

        def horiz_bank(bank, st, k):
            q0 = bank * HPERBANK
            pt = psum.tile([128, HSLOT * NH], F32, tag="hp")
            for g, base in enumerate(HBASES):
                for j in range(HSLOT):
                    q = q0 + g * HSLOT + j
                    h, i = divmod(q, NCH)
                    w0 = i * MH
                    nc.tensor.matmul(
                        out=pt[base:base + MH, j * NH:(j + 1) * NH],
                        lhsT=src3[:, h, w0:w0 + MH],
                        rhs=tgt3[:, h + R, w0:w0 + NH],
                        start=True, stop=True,
                        tile_position=(0, base),
                    )
            seg = HSLOT * NH
            stage_copy(st[:, k * seg:(k + 1) * seg], pt)

        def horiz_flush(st, grp):
            nb, b0 = len(grp), grp[0]
            seg = HSLOT * NH
            for g, base in enumerate(HBASES):
                nc.scalar.dma_start(
                    out=hband[:][:, g, b0 * HSLOT:(b0 + nb) * HSLOT, :],
                    in_=st[base:base + MH, :nb * seg],
                )

        def v_ng(bank, g):
            return min(VSLOT, max(0, min(VPERBANK, W - bank * VPERBANK) - g * VSLOT))

        def vert_bank(bank, st, k):
            w0 = bank * VPERBANK
            pt = psum.tile([112, VSLOT * HT], F32, tag="vp")
            for g, base in enumerate(VBASES):
                for j in range(v_ng(bank, g)):
                    w = w0 + g * VSLOT + j
                    nc.tensor.matmul(
                        out=pt[base:base + HSH, j * HT:(j + 1) * HT],
                        lhsT=src3[:, :, w],
                        rhs=tgt3[:, 0:HT, w + R],
                        start=True, stop=True,
                        tile_position=(0, base),
                    )
            seg = VSLOT * HT
            for g, base in enumerate(VBASES):
                ng = v_ng(bank, g)
                if ng > 0:
                    stage_copy(
                        st[base:base + HSH, k * seg:k * seg + ng * HT],
                        pt[base:base + HSH, :ng * HT],
                    )

        def vert_flush(st, gi):
            nc.scalar.dma_start(out=vband[:][gi], in_=st[0:112, :])

        HGRP = [[0, 1, 2], [3, 4]]
        VGRP = [[0, 1, 2], [3, 4, 5], [6, 7, 8]]
        hseg, vseg = HSLOT * NH, VSLOT * HT

        # pipeline: issue loads piecewise; horizontal banks unlock as the
        # rows they need land; vertical banks need all pieces.
        load_piece(0)
        load_piece(1)
        hst = stage.tile([128, 3 * hseg], bdt, tag="hs")
        horiz_bank(0, hst, 0)
        load_piece(2)
        horiz_bank(1, hst, 1)
        load_piece(3)
        horiz_bank(2, hst, 2)
        horiz_flush(hst, HGRP[0])
        hst2 = stage.tile([128, 3 * hseg], bdt, tag="hs")
        horiz_bank(3, hst2, 0)
        horiz_bank(4, hst2, 1)
        horiz_flush(hst2, HGRP[1])
        for gi, grp in enumerate(VGRP):
            vst = stage.tile([112, 3 * vseg], bdt, tag="vs")
            # zero hole partitions (48:64) + group-B rows so one whole-tile
            # DMA per group is fully initialized; copies overwrite the rest
            nc.gpsimd.memset(vst, 0.0)
            for k, bank in enumerate(grp):
                vert_bank(bank, vst, k)
            vert_flush(vst, gi)

    nc.compile()
    return nc


_NC_CACHE = []


def _get_nc():
    if not _NC_CACHE:
        _NC_CACHE.append(build_nc())
    return _NC_CACHE[0]


def shard_inputs(src, tgt):
    if COMPUTE_DT == "float32":
        np_cdt = np.float32
    elif COMPUTE_DT == "float16":
        np_cdt = np.float16
    else:
        import ml_dtypes
        np_cdt = np.dtype(ml_dtypes.bfloat16)
    src = np.asarray(src, dtype=np.float32)
    tgt = np.asarray(tgt, dtype=np.float32)
    tp = np.pad(tgt, ((0, 0), (0, 0), (R, R), (R, R)))
    in_maps = []
    for core in range(NCORES):
        b, hh = divmod(core, 2)
        h0 = hh * HSH
        s = np.ascontiguousarray(src[b, :, h0:h0 + HSH, :]).reshape(C, HSH * W)
        t = np.ascontiguousarray(tp[b, :, h0:h0 + HT, :]).reshape(C, HT * WP)
        in_maps.append({"src": s.astype(np_cdt), "tgt": t.astype(np_cdt)})
    return in_maps


def extract_output(results):
    """results: list of 8 dicts with
    'vband' [2, 48, NVBANK, 9, 56], 'hband' [32, 4, NHBANK*12, 40]."""
    out = np.zeros((B, len(SHIFTS), H, W), np.float32)
    hidx = np.arange(HSH)
    midx = np.arange(MH)
    widx = np.arange(W)
    iidx = np.arange(NCH)
    for core in range(NCORES):
        b, hh = divmod(core, 2)
        h0 = hh * HSH
        # [grp, p, bank, j*56+h'] -> [h, w=grp*54+bank*18+g*9+j, h']
        # where p = 64*g + h (partition groups at 0 and 64, holes 48:64)
        vb = np.asarray(results[core]["vband"]).astype(np.float32)
        vb = vb.reshape(3, 112, 3, VSLOT, HT)
        vb = np.stack([vb[:, 0:HSH], vb[:, 64:64 + HSH]], axis=3)
        vb = vb.transpose(1, 0, 2, 3, 4, 5).reshape(HSH, 162, HT)[:, :W, :]
        # [m,g,bank*12+j,n] -> [m, q=bank*48+g*12+j, n] -> [m,h,i,n]
        hb = np.asarray(results[core]["hband"]).astype(np.float32)
        hb = hb.reshape(MH, len(HBASES), NHBANK, HSLOT, NH)
        hb = hb.transpose(0, 2, 1, 3, 4).reshape(MH, NQ, NH)
        hb = hb.reshape(MH, HSH, NCH, NH)
        for s, (dh, dw) in enumerate(SHIFTS):
            if dw == 0:
                out[b, s, h0:h0 + HSH, :] = vb[
                    hidx[:, None], widx[None, :], (hidx + dh + R)[:, None]
                ]
            else:
                v = hb[
                    midx[:, None, None],
                    hidx[None, :, None],
                    iidx[None, None, :],
                    (midx + dw + R)[:, None, None],
                ]  # [m, h, i]
                out[b, s, h0:h0 + HSH, :] = v.transpose(1, 2, 0).reshape(HSH, W)
    return out


def kernel(src, tgt, **run_kwargs):
    nc = _get_nc()
    in_maps = shard_inputs(src, tgt)
    res = bass_utils.run_bass_kernel_spmd(
        nc, in_maps, core_ids=list(range(NCORES)), **run_kwargs
    )
    out = extract_output(res.results)
    kernel.last_result = res
    return out



# revision 5
# speedup vs baseline: 1.2814x; 1.2814x over previous
"""Cost-volume layer (17-shift cross pattern, R=4) for Trainium2, 8 NeuronCores.

out[b,s,h,w] = sum_c src[b,c,h,w] * tgt[b,c,h+dh_s,w+dw_s]   (tgt zero-padded)

Strategy (column-progressive pipeline)
--------------------------------------
Shard: 8 cores = batch(4) x H-halves(2). Per core the inputs are HOST-
TRANSPOSED to w-major: src [C, W=160, 48], tgt [C, 160, 56] (8-row H halo,
W pad added on device via memset strips). C=128 is the SBUF partition dim,
contracted on the TensorEngine via banded correlations:

- vertical shifts  (dh=-4..4): per column w, matmul
    src[:, w, :]^T @ tgt[:, w+4, :]  ->  [48, 56] band
- horizontal shifts (dw=-4..4): per row h, per 32-col chunk i, matmul
    src[:, 32i:32i+32, h]^T @ tgt[:, 32i:32i+40, h+4] -> [32, 40] band

Because the layout is w-major, BOTH passes unlock column-piece by column-
piece: inputs load as 10 x 16-col pieces (tgt on the sync HWDGE ring, src
on the scalar ring, issued up front so the SDMA engines stream back to
back at full HBM rate), and the PE consumes V-bank(p) / H-bank(i) work the
moment piece p lands. The PE is never idle, which keeps the p-state ramp
at full clock. PSUM bands are staged to SBUF with full-partition-width
copies round-robined over DVE/ACT/Pool, and flushed to HBM on the sync
ring (which is idle after the input issues) so no compute sequencer ever
blocks on a DMA semaphore. Diagonals are gathered host-side from the
bands (pure indexing, no host arithmetic).
"""

import numpy as np
from contextlib import ExitStack

import concourse.bacc as bacc
import concourse.tile as tile
from concourse import mybir
from concourse import bass_utils

R = 4
B, C, H, W = 4, 128, 96, 160
NCORES = 8
HSH = H // 2            # 48 output rows per shard
HT = HSH + 2 * R        # 56 tgt rows (with halo)
WT = W + 2 * R          # 168 padded width (device)
F32 = mybir.dt.float32
F16 = mybir.dt.float16

PC = 16                 # piece width (cols)
NP = W // PC            # 10 pieces
# vertical: bank p covers w in [16p, 16p+16); w = 16p + 8g + s,
# g in {0,1} -> PSUM partition base 64g (holes 48:64), s in 0..8
VS = 8                  # slots per group
# horizontal: bank i covers chunks (h, i), h in 0..47; partition base
# 32*(h%4), slot h//4 in 0..11
MH = 32
NH = MH + 2 * R         # 40
NCH = W // MH           # 5 chunk columns = 5 H banks
HSLOT = HSH // 4        # 12

SHIFTS = [(0, 0)]
for i in range(1, R + 1):
    SHIFTS.extend([(-i, 0), (i, 0), (0, -i), (0, i)])


def build_nc():
    nc = bacc.Bacc("TRN2", target_bir_lowering=False)
    src = nc.dram_tensor("src", [C, W * HSH], F16, kind="ExternalInput")
    tgt = nc.dram_tensor("tgt", [C, W * HT], F16, kind="ExternalInput")
    # vband[gi] = V banks {2gi, 2gi+1}: [112, 2, 8, 56] (partition 64g+hh)
    vband = nc.dram_tensor("vband", [NP // 2, 112, 2 * VS * HT], F16,
                           kind="ExternalOutput")
    # hband[i] = H bank i: [128, 12, 40] (partition 32*(h%4)+m)
    hband = nc.dram_tensor("hband", [NCH, 128, HSLOT * NH], F16,
                           kind="ExternalOutput")

    with ExitStack() as ctx:
        tc = ctx.enter_context(tile.TileContext(nc))
        ins = ctx.enter_context(tc.tile_pool(name="ins", bufs=1))
        psum = ctx.enter_context(tc.tile_pool(name="psum", bufs=1, space="PSUM"))
        stage = ctx.enter_context(tc.tile_pool(name="stage", bufs=3))

        src_sb = ins.tile([C, W * HSH], F16)
        tgt_sb = ins.tile([C, WT * HT], F16)
        src3 = src_sb.rearrange("c (w h) -> c w h", h=HSH)
        tgt3 = tgt_sb.rearrange("c (w r) -> c w r", r=HT)
        srcd = src[:].rearrange("c (w h) -> c w h", h=HSH)
        tgtd = tgt[:].rearrange("c (w r) -> c w r", r=HT)

        # zero the W-pad strips (cols 0:4 and 164:168 of padded tgt)
        nc.gpsimd.memset(tgt3[:, 0:R, :], 0.0)
        nc.gpsimd.memset(tgt3[:, R + W:WT, :], 0.0)

        # issue ALL input piece loads up front: tgt on sync, src on scalar.
        # SDMA streams them back-to-back; compute unlocks per piece via
        # Tile's subtile dependency tracking.
        for p in range(NP):
            nc.sync.dma_start(
                out=tgt3[:, R + PC * p:R + PC * (p + 1), :],
                in_=tgtd[:, PC * p:PC * (p + 1), :],
            )
        for p in range(NP):
            nc.scalar.dma_start(
                out=src3[:, PC * p:PC * (p + 1), :],
                in_=srcd[:, PC * p:PC * (p + 1), :],
            )

        copy_flip = [0]

        def stage_copy(dst, src_ap):
            # GPSIMD cannot access PSUM on TRN2 -> DVE/ACT only
            eng = (nc.vector.tensor_copy, nc.scalar.copy)[copy_flip[0] % 2]
            eng(out=dst, in_=src_ap)
            copy_flip[0] += 1

        vseg = 2 * VS * HT      # 896

        def vert_bank(p, st, half):
            pt = psum.tile([112, VS * HT], F32, tag="vp", bufs=3)
            for g in range(2):
                for s in range(VS):
                    w = PC * p + VS * g + s
                    nc.tensor.matmul(
                        out=pt[64 * g:64 * g + HSH, s * HT:(s + 1) * HT],
                        lhsT=src3[:, w, :],
                        rhs=tgt3[:, w + R, :],
                        start=True, stop=True,
                        tile_position=(0, 64 * g),
                    )
            # copy both groups (partitions 0:112; 48:64 are unwritten holes
            # the host ignores) in one full-width instruction
            stage_copy(st[:, half * (VS * HT):(half + 1) * (VS * HT)], pt)

        def horiz_bank(i, st):
            pt = psum.tile([128, HSLOT * NH], F32, tag="hp", bufs=2)
            for h in range(HSH):
                base = 32 * (h % 4)
                j = h // 4
                nc.tensor.matmul(
                    out=pt[base:base + MH, j * NH:(j + 1) * NH],
                    lhsT=src3[:, MH * i:MH * (i + 1), h],
                    rhs=tgt3[:, MH * i:MH * i + NH, h + R],
                    start=True, stop=True,
                    tile_position=(0, base),
                )
            stage_copy(st, pt)

        # schedule: V banks as pieces land; H bank i after piece 2i+2
        # (it needs tgt cols [32i, 32i+40) = pieces up to 2i+2 plus strips)
        vst = None
        hdone = 0
        for p in range(NP):
            if p % 2 == 0:
                vst = stage.tile([112, vseg], F16, tag="vs", name=f"vst{p}")
            vert_bank(p, vst, p % 2)
            if p % 2 == 1:
                nc.sync.dma_start(out=vband[:][p // 2], in_=vst)
            # H banks interleave: after piece 2,4,6,8 and the last piece
            while hdone < NCH and (p >= 2 * hdone + 2 or p == NP - 1):
                hst = stage.tile([128, HSLOT * NH], F16, tag="hs",
                                 name=f"hst{hdone}")
                horiz_bank(hdone, hst)
                nc.sync.dma_start(out=hband[:][hdone], in_=hst)
                hdone += 1

    nc.compile()
    return nc


_NC_CACHE = []


def _get_nc():
    if not _NC_CACHE:
        _NC_CACHE.append(build_nc())
    return _NC_CACHE[0]


def shard_inputs(src, tgt):
    src = np.asarray(src, dtype=np.float32)
    tgt = np.asarray(tgt, dtype=np.float32)
    tp = np.pad(tgt, ((0, 0), (0, 0), (R, R), (0, 0)))  # pad H only
    in_maps = []
    for core in range(NCORES):
        b, hh = divmod(core, 2)
        h0 = hh * HSH
        s = src[b, :, h0:h0 + HSH, :].transpose(0, 2, 1)       # [C, W, 48]
        t = tp[b, :, h0:h0 + HT, :].transpose(0, 2, 1)         # [C, W, 56]
        in_maps.append({
            "src": np.ascontiguousarray(s).reshape(C, W * HSH).astype(np.float16),
            "tgt": np.ascontiguousarray(t).reshape(C, W * HT).astype(np.float16),
        })
    return in_maps


def extract_output(results):
    """results: per core 'vband' [5, 112, 896], 'hband' [5, 128, 480]."""
    out = np.zeros((B, len(SHIFTS), H, W), np.float32)
    hidx = np.arange(HSH)
    midx = np.arange(MH)
    for core in range(NCORES):
        b, hh = divmod(core, 2)
        h0 = hh * HSH
        vb = np.asarray(results[core]["vband"]).astype(np.float32)
        vb = vb.reshape(NP // 2, 112, 2, VS, HT)   # [gi, part, half, s, r]
        # part = 64g + hh' (holes 48:64); w = 16*(2gi+half) + 8g + s
        vbg = np.stack([vb[:, 0:HSH], vb[:, 64:64 + HSH]], axis=1)
        # [gi, g, hh', half, s, r] -> [gi, half, g, s, hh', r] -> [w, hh', r]
        vbw = vbg.transpose(0, 3, 1, 4, 2, 5).reshape(W, HSH, HT)
        hb = np.asarray(results[core]["hband"]).astype(np.float32)
        hb = hb.reshape(NCH, 4, MH, HSLOT, NH)     # [i, h%4, m, h//4, n]
        hb = hb.transpose(3, 1, 0, 2, 4).reshape(HSH, NCH, MH, NH)
        for si, (dh, dw) in enumerate(SHIFTS):
            if dw == 0:
                v = vbw[:, hidx, hidx + dh + R]        # [W, 48]
                out[b, si, h0:h0 + HSH, :] = v.T
            else:
                v = hb[:, :, midx, midx + dw + R]      # [48, 5, 32]
                out[b, si, h0:h0 + HSH, :] = v.reshape(HSH, W)
    return out


def kernel(src, tgt, **run_kwargs):
    nc = _get_nc()
    in_maps = shard_inputs(src, tgt)
    res = bass_utils.run_bass_kernel_spmd(
        nc, in_maps, core_ids=list(range(NCORES)), **run_kwargs
    )
    out = extract_output(res.results)
    kernel.last_result = res
    return out
